# revision 45
# baseline (speedup 1.0000x reference)
"""MLA (multi-head latent attention) forward on 8 Trainium2 NeuronCores.

Sharding: 2 groups of 4 cores, one batch element per group. Within a group:
- kv+krope compress is token-parallel (2x256-token blocks/core), exchanged via
  two chunked AllGathers (576 rows x 256 tokens each) that overlap compute.
- q compress is replicated (each core compresses all 2048 batch tokens), so q
  latents never leave the core.
- decompress + attention are head-parallel (4 heads/core).
- output projection is computed as per-supertile partials over the core's own
  heads and combined with pipelined ReduceScatter(add) collectives.
RoPE via duplicated-weight columns (no repacking DMAs); causal diagonal
narrowing on attention tiles.
"""

import numpy as np

import concourse.bacc as bacc
import concourse.mybir as mybir
import concourse.tile as tile
from concourse import bass_utils

B, S, D = 2, 2048, 2048
H = 16
NOPE, ROPE, VH = 128, 64, 128
QR = KVR = 512
EPS = 1e-6
W = 8            # cores
GS = 4           # group size (cores per batch)
HPC = H // GS    # heads per core = 4
TBLK = 256       # compress block tokens
SQ = 512         # q supertile width
KT = 128         # key tile
NST = S // SQ    # 4 supertiles
NBLK = S // TBLK # 8 blocks per batch
SCALE = 1.0 / np.sqrt(NOPE + ROPE)
KVROWS = KVR + ROPE  # 576 latent rows shipped (kv 0:512, krope 512:576)

FP16 = mybir.dt.float16
FP32 = mybir.dt.float32

_cache = {}


def _build():
    nc = bacc.Bacc("TRN2", target_bir_lowering=False, debug=False)

    def din(name, shape, dt=FP16):
        return nc.dram_tensor(name, shape, dt, kind="ExternalInput").ap()

    xT = din("xT", [D, S])                 # full batch, feature-major
    xmyT = din("xmyT", [D, 2 * TBLK])      # my 2 blocks, feature-major
    wcmpkv = din("wcmpkv", [D, 640])       # w_ckvT | w_kropeT_dup
    wcmpq = din("wcmpq", [D, 512])         # w_cqT
    wdec = din("wdec", [QR, 2048])         # wdqnT | wdqr_dupT | wdknT | wdvT (4 heads)
    wprojT = din("wprojT", [HPC * VH, D])  # my heads' rows of w_proj.T
    ropeq = din("ropeq", [128, S])         # [cos;sin;-sin;cos] all positions
    ropek = din("ropek", [128, 2 * TBLK])  # same, my 2 blocks' positions
    eye64 = din("eye64", [128, 64])        # [I64; I64]
    tril = din("tril", [128, 128])         # tril[k,q] = (q >= k)
    out_c = nc.dram_tensor("out_c", [SQ, D], FP16, kind="ExternalOutput").ap()

    groups = [[0, 1, 2, 3], [4, 5, 6, 7]]

    with tile.TileContext(nc) as tc:
        dram_cm = tc.tile_pool(name="dram", bufs=1, space="DRAM")
        dram = dram_cm.__enter__()
        lat_a = dram.tile([KVROWS, TBLK], FP16, tag="lat_a", name="lat_a")
        lat_b = dram.tile([KVROWS, TBLK], FP16, tag="lat_b", name="lat_b")
        lat_ga = dram.tile([GS, KVROWS, TBLK], FP16, tag="lat_ga", name="lat_ga")
        lat_gb = dram.tile([GS, KVROWS, TBLK], FP16, tag="lat_gb", name="lat_gb")
        rs_in = dram.tile([NST, SQ, D], FP16, tag="rs_in", name="rs_in")
        rs_out = dram.tile([NST, 128, D], FP16, tag="rs_out", name="rs_out")

        const_cm = tc.tile_pool(name="const", bufs=1)
        const = const_cm.__enter__()
        ones_h = const.tile([128, 128], FP16, tag="ones_h", name="ones_h")
        nc.any.memset(ones_h[:], 1.0)
        ones1 = const.tile([1, 128], FP16, tag="ones1", name="ones1")
        nc.any.memset(ones1[:], 1.0)
        invn = const.tile([128, 1], FP16, tag="invn", name="invn")
        nc.any.memset(invn[:], 1.0 / QR)
        eps_t = const.tile([1, 1], FP32, tag="eps_t", name="eps_t")
        nc.any.memset(eps_t[:], EPS)
        tril_t = const.tile([128, 128], FP16, tag="tril_t", name="tril_t")
        nc.sync.dma_start(tril_t[:], tril[:])
        eye_t = const.tile([128, 64], FP16, tag="eye_t", name="eye_t")
        nc.sync.dma_start(eye_t[:], eye64[:])
        ropeq_t = const.tile([128, S], FP16, tag="ropeq_t", name="ropeq_t")
        nc.sync.dma_start(ropeq_t[:], ropeq[:])
        ropek_t = const.tile([128, 2 * TBLK], FP16, tag="ropek_t", name="ropek_t")
        nc.sync.dma_start(ropek_t[:], ropek[:])

        # persistent q latents (replicated compress)
        nq_cm = tc.tile_pool(name="nq", bufs=1)
        nqp = nq_cm.__enter__()
        nq_all = [nqp.tile([128, S], FP16, tag=f"nq{k}", name=f"nq{k}") for k in range(4)]

        # dec-phase pools allocated BEFORE compress pools so their SBUF/PSUM
        # space is disjoint (avoids false WAR waits on compress tiles)
        dec_w_cm = tc.tile_pool(name="dec_w", bufs=1)
        dec_w = dec_w_cm.__enter__()
        dec_s_cm = tc.tile_pool(name="dec_s", bufs=1)
        dec_s = dec_s_cm.__enter__()
        ps_lin_cm = tc.tile_pool(name="ps_lin", bufs=2, space="PSUM")
        ps_lin = ps_lin_cm.__enter__()
        wd = [dec_w.tile([128, 2048], FP16, tag=f"wd{k}", name=f"wd{k}") for k in range(4)]
        wp = [dec_w.tile([128, D], FP16, tag=f"wp{k}", name=f"wp{k}") for k in range(4)]

        # ---------------- Phase 1: compress ------------------------------------
        with tc.tile_pool(name="cmp_w", bufs=1) as cmp_w, \
             tc.tile_pool(name="cmp_t", bufs=1) as cmp_t, \
             tc.tile_pool(name="ps_cmp", bufs=1, space="PSUM") as ps_cmp:
            # kv path needs wckv and xmy first -- load those on SP.
            wckv = cmp_w.tile([128, 16 * 640], FP16, tag="wckv", name="wckv")
            xmy = cmp_w.tile([128, 16 * 2 * TBLK], FP16, tag="xmy", name="xmy")
            for hh in range(8):
                ks = slice(hh * 2 * 640, (hh + 1) * 2 * 640)
                nc.sync.dma_start(wckv[:, ks].rearrange("p (k c) -> p k c", k=2),
                                  wcmpkv[hh * 256:(hh + 1) * 256, :].rearrange("(k p) c -> p k c", p=128))
                ks = slice(hh * 2 * 512, (hh + 1) * 2 * 512)
                nc.sync.dma_start(xmy[:, ks].rearrange("p (k t) -> p k t", k=2),
                                  xmyT[hh * 256:(hh + 1) * 256, :].rearrange("(k p) t -> p k t", p=128))
            wcq = cmp_w.tile([128, 16 * 512], FP16, tag="wcq", name="wcq")
            xtc = {}

            def load_xt(cc):
                t = cmp_w.tile([128, 16 * 512], FP16, tag="xt", name=f"xt{cc}", bufs=2)
                xtc[cc] = t
                for hh in range(4):
                    ks = slice(hh * 4 * 512, (hh + 1) * 4 * 512)
                    nc.sync.dma_start(
                        t[:, ks].rearrange("p (k t) -> p k t", k=4),
                        xT[hh * 512:(hh + 1) * 512, cc * 512:(cc + 1) * 512].rearrange(
                            "(k p) t -> p k t", p=128))

            load_xt(0)
            for hh in range(4):
                ks = slice(hh * 4 * 512, (hh + 1) * 4 * 512)
                nc.sync.dma_start(wcq[:, ks].rearrange("p (k c) -> p k c", k=4),
                                  wcmpq[hh * 512:(hh + 1) * 512, :].rearrange("(k p) c -> p k c", p=128))


            def rmsnorm(ps4, dst_fn):
                """ps4: 4 psum APs [128, TBLK]; dst_fn(m) -> fp16 dest AP."""
                cq = cmp_t.tile([128, 4 * TBLK], FP32, tag="cq", name="cq", bufs=2)
                sq = cmp_t.tile([128, 4 * TBLK], FP16, tag="sq", name="sq", bufs=2)
                for m in range(4):
                    cs = slice(m * TBLK, (m + 1) * TBLK)
                    nc.scalar.activation(cq[:, cs], ps4[m], mybir.ActivationFunctionType.Copy)
                    nc.vector.tensor_mul(sq[:, cs], cq[:, cs], cq[:, cs])
                # one psum bank: ssq in [0:1, 0:TBLK], rstd broadcast in [:, TBLK:2*TBLK]
                ps_nrm = ps_cmp.tile([128, 2 * TBLK], FP32, tag="nrm", name="nrm", bufs=1)
                for m in range(4):
                    nc.tensor.matmul(ps_nrm[0:1, 0:TBLK], invn[:], sq[:, m * TBLK:(m + 1) * TBLK],
                                     start=(m == 0), stop=(m == 3))
                std_f = cmp_t.tile([1, TBLK], FP32, tag="std", name="std", bufs=2)
                nc.scalar.activation(std_f[:], ps_nrm[0:1, 0:TBLK], mybir.ActivationFunctionType.Sqrt,
                                     bias=eps_t[:])
                rstd_f = cmp_t.tile([1, TBLK], FP32, tag="rstdf", name="rstdf", bufs=2)
                nc.vector.reciprocal(rstd_f[:], std_f[:])
                rstd = cmp_t.tile([1, TBLK], FP16, tag="rstd", name="rstd", bufs=2)
                nc.vector.tensor_copy(rstd[:], rstd_f[:])
                nc.tensor.matmul(ps_nrm[:, TBLK:2 * TBLK], ones1[:], rstd[:], start=True, stop=True)
                for m in range(4):
                    nc.vector.tensor_mul(dst_fn(m), cq[:, m * TBLK:(m + 1) * TBLK],
                                         ps_nrm[:, TBLK:2 * TBLK])

            # kv+krope for my 2 blocks -> AllGather chunks
            def kv_block(half):
                lat_dst, lat_g = ((lat_a, lat_ga), (lat_b, lat_gb))[half]
                xoff = half * TBLK  # column offset inside my 2-block slice (input-dependent)
                psk = [ps_cmp.tile([128, TBLK], FP32, tag=f"cmk{m}", name=f"cmk{m}")[:]
                       for m in range(4)]
                psr = ps_cmp.tile([128, TBLK], FP32, tag="cmr", name="cmr")[:]
                for k in range(16):
                    xk = xmy[:, k * 2 * TBLK + xoff: k * 2 * TBLK + xoff + TBLK]
                    wb = k * 640
                    for m in range(4):
                        nc.tensor.matmul(psk[m], wckv[:, wb + m * 128: wb + (m + 1) * 128],
                                         xk, start=(k == 0), stop=(k == 15))
                    nc.tensor.matmul(psr, wckv[:, wb + 512: wb + 640],
                                     xk, start=(k == 0), stop=(k == 15))

                lat_s = cmp_t.tile([128, 4 * TBLK], FP16, tag="lat_s", name="lat_s", bufs=2)
                rmsnorm(psk, lambda m: lat_s[:, m * TBLK:(m + 1) * TBLK])
                nc.sync.dma_start(
                    lat_dst[0:KVR, :].rearrange("(m p) c -> p m c", p=128),
                    lat_s[:].rearrange("p (m c) -> p m c", m=4))
                # krope: rope via dup layout, fold 128->64 rows with [I;I] matmul
                ktmp = cmp_t.tile([128, TBLK], FP16, tag="ktmp", name="ktmp", bufs=2)
                nc.vector.tensor_mul(ktmp[:], psr,
                                     ropek_t[:, half * TBLK:(half + 1) * TBLK])
                ps_nrm_f = ps_cmp.tile([128, 2 * TBLK], FP32, tag="nrm", name="nrm", bufs=1)
                ps_kr = ps_nrm_f[64:128, 0:TBLK]
                nc.tensor.matmul(ps_kr, eye_t[:], ktmp[:], start=True, stop=True)
                kr16 = cmp_t.tile([64, TBLK], FP16, tag="kr16", name="kr16", bufs=2)
                nc.scalar.activation(kr16[:], ps_kr, mybir.ActivationFunctionType.Copy)
                nc.sync.dma_start(lat_dst[KVR:KVROWS, :], kr16[:])

                nc.gpsimd.collective_compute(
                    "AllGather",
                    mybir.AluOpType.bypass,
                    ins=[lat_dst[:].rearrange("a b -> (a b)")],
                    outs=[lat_g[:].rearrange("w a b -> (w a b)")],
                    replica_groups=groups,
                )

            kv_block(0)
            load_xt(1)
            kv_block(1)
            for k in range(4):
                nc.sync.dma_start(wd[k][:], wdec[k * 128:(k + 1) * 128, :])
            for k in range(4):
                nc.sync.dma_start(wp[k][:], wprojT[k * 128:(k + 1) * 128, :])

            # q compress for ALL 8 blocks (replicated) -> nq_all sbuf
            for blk in range(NBLK):
                if blk in (2, 4) and blk // 2 + 1 < 4:
                    load_xt(blk // 2 + 1)
                bs = slice(blk * TBLK, (blk + 1) * TBLK)
                xsrc = xtc[blk // 2]
                xo = (blk % 2) * TBLK
                psq = [ps_cmp.tile([128, TBLK], FP32, tag=f"cmk{m}", name=f"cmk{m}")[:]
                       for m in range(4)]
                for k in range(16):
                    xk = xsrc[:, k * 512 + xo: k * 512 + xo + TBLK]
                    for m in range(4):
                        nc.tensor.matmul(psq[m], wcq[:, k * 512 + m * 128: k * 512 + (m + 1) * 128],
                                         xk, start=(k == 0), stop=(k == 15))
                rmsnorm(psq, lambda m, bs=bs: nq_all[m][:, bs])

        # ---------------- Phase 2: decompress + attention + partial proj ---------
        attn_cm = tc.tile_pool(name="attn", bufs=1)
        ap_ = attn_cm.__enter__()
        qn = [ap_.tile([128, S], FP16, tag=f"qn{h}", name=f"qn{h}") for h in range(HPC)]
        qr_ = [ap_.tile([128, S], FP16, tag=f"qr{h}", name=f"qr{h}") for h in range(HPC)]
        kn = [ap_.tile([128, S], FP16, tag=f"kn{h}", name=f"kn{h}") for h in range(HPC)]
        krg2 = [dec_s.tile([128, SQ], FP16, tag=f"krg2{st}", name=f"krg2{st}")
                for st in range(NST)]
        val = ap_.tile([128, 16 * 512], FP16, tag="val", name="val")

        with tc.tile_pool(name="ps_s", bufs=2, space="PSUM") as ps_s, \
             tc.tile_pool(name="ps_av", bufs=2, space="PSUM") as ps_av, \
             tc.tile_pool(name="ps_z", bufs=2, space="PSUM") as ps_z, \
             tc.tile_pool(name="att_t", bufs=1) as att_t:
            # preload ALL kv latent chunks up front (SP blocks only on the AGs)
            nkv_all = []
            for st in range(NST):
                lat_g = lat_ga if st < 2 else lat_gb
                j0 = (st % 2) * 2
                ss = slice(st * SQ, (st + 1) * SQ)
                nkv_t = []
                for k in range(4):
                    t_ = dec_s.tile([128, SQ], FP16, tag=f"nkv{st}{k}", name=f"nkv{st}{k}")
                    nc.sync.dma_start(
                        t_[:].rearrange("p (j c) -> p j c", j=2),
                        lat_g[j0:j0 + 2, k * 128:(k + 1) * 128, :].rearrange("j p c -> p j c"))
                    nkv_t.append(t_)
                nkv_all.append(nkv_t)
                nc.sync.dma_start(
                    krg2[st][0:64, :].rearrange("p (j c) -> p j c", j=2),
                    lat_g[j0:j0 + 2, KVR:KVROWS, :].rearrange("j p c -> p j c"))
                nc.sync.dma_start(
                    krg2[st][64:128, :].rearrange("p (j c) -> p j c", j=2),
                    lat_g[j0:j0 + 2, KVR:KVROWS, :].rearrange("j p c -> p j c"))

            def emit_rs(st):
                nc.gpsimd.collective_compute(
                    "ReduceScatter",
                    mybir.AluOpType.add,
                    ins=[rs_in[st].rearrange("a b -> (a b)")],
                    outs=[rs_out[st].rearrange("a b -> (a b)")],
                    replica_groups=groups,
                )
                nc.sync.dma_start(out_c[st * 128:(st + 1) * 128, :], rs_out[st])

            def emit_proj(st, ao):
                # partial output projection for supertile st's tokens
                stage = att_t.tile([128, 4 * D], FP16, tag="stage", name="stage", bufs=2)
                for tch in range(4):
                    tsl = slice(tch * 128, (tch + 1) * 128)
                    for dch in range(4):
                        ps = ps_lin.tile([128, 512], FP32, tag="lin", name="lin")
                        for h in range(HPC):
                            nc.tensor.matmul(ps[:], ao[h][:, tsl],
                                             wp[h][:, dch * 512:(dch + 1) * 512],
                                             start=(h == 0), stop=(h == 3))
                        if (tch + dch) % 2 == 0:
                            nc.scalar.activation(stage[:, tch * D + dch * 512: tch * D + (dch + 1) * 512],
                                                 ps[:], mybir.ActivationFunctionType.Copy)
                        else:
                            nc.vector.tensor_copy(stage[:, tch * D + dch * 512: tch * D + (dch + 1) * 512],
                                                  ps[:])
                for hh in range(2):
                    nc.sync.dma_start(
                        rs_in[st, hh * 256:(hh + 1) * 256, :].rearrange("(t p) d -> p t d", p=128),
                        stage[:, hh * 2 * D:(hh + 1) * 2 * D].rearrange("p (t d) -> p t d", t=2))
                emit_rs(st)

            prev_proj = None
            for st in range(NST):
                ss = slice(st * SQ, (st + 1) * SQ)
                nkv_t = nkv_all[st]

                # q decompress from local latents (no AG dependency)
                for h in range(HPC):
                    ps = ps_lin.tile([128, SQ], FP32, tag="lin", name="lin")
                    for k in range(4):
                        nc.tensor.matmul(ps[:], wd[k][:, h * 128:(h + 1) * 128],
                                         nq_all[k][:, ss], start=(k == 0), stop=(k == 3))
                    nc.vector.tensor_copy(qn[h][:, ss], ps[:])
                for h in range(HPC):
                    ps = ps_lin.tile([128, SQ], FP32, tag="lin", name="lin")
                    for k in range(4):
                        nc.tensor.matmul(ps[:], wd[k][:, 512 + h * 128: 512 + (h + 1) * 128],
                                         nq_all[k][:, ss], start=(k == 0), stop=(k == 3))
                    nc.vector.tensor_mul(qr_[h][:, ss], ps[:], ropeq_t[:, ss])

                # k/v decompress
                for h in range(HPC):
                    ps = ps_lin.tile([128, SQ], FP32, tag="lin", name="lin")
                    for k in range(4):
                        nc.tensor.matmul(ps[:], wd[k][:, 1024 + h * 128: 1024 + (h + 1) * 128],
                                         nkv_t[k][:], start=(k == 0), stop=(k == 3))
                    nc.scalar.activation(kn[h][:, ss], ps[:], mybir.ActivationFunctionType.Copy)
                for j in range(4):
                    ps = ps_lin.tile([128, SQ], FP32, tag="lin", name="lin")
                    for k in range(4):
                        nc.tensor.matmul(ps[:], nkv_t[k][:, j * 128:(j + 1) * 128],
                                         wd[k][:, 1536:2048], start=(k == 0), stop=(k == 3))
                    ktg = st * 4 + j
                    nc.vector.tensor_copy(val[:, ktg * 512:(ktg + 1) * 512], ps[:])

                # previous supertile's projection (hides softmax-normalize latency)
                if prev_proj is not None:
                    emit_proj(*prev_proj)

                # attention (Q = st), 4 heads
                nkt = 4 * (st + 1)
                ao = []
                for h in range(HPC):
                    pav = ps_av.tile([128, SQ], FP32, tag="av", name="av")
                    pz = ps_z.tile([128, SQ], FP32, tag="z", name="z")
                    z_started = False
                    zacc = None  # running sum of full-tile pT's
                    for kt in range(nkt):
                        d = kt - 4 * st
                        off = max(0, d) * 128
                        ncols = SQ - off
                        ks = slice(kt * KT, (kt + 1) * KT)
                        qs = slice(st * SQ + off, (st + 1) * SQ)
                        pss = ps_s.tile([128, SQ], FP32, tag="s", name="s")
                        nc.tensor.matmul(pss[:, 0:ncols], kn[h][:, ks], qn[h][:, qs],
                                         start=True, stop=False)
                        nc.tensor.matmul(pss[:, 0:ncols], krg2[kt // 4][:, (kt % 4) * KT:(kt % 4 + 1) * KT], qr_[h][:, qs],
                                         start=False, stop=True)
                        pT = att_t.tile([128, SQ], FP16, tag="pT", name="pT", bufs=6)
                        nc.scalar.activation(pT[:, 0:ncols], pss[:, 0:ncols],
                                             mybir.ActivationFunctionType.Exp)
                        if d >= 0:
                            nc.vector.tensor_mul(pT[:, 0:128], pT[:, 0:128], tril_t[:])
                        vs = slice(kt * 512 + h * 128, kt * 512 + (h + 1) * 128)
                        nc.tensor.matmul(pav[:, off:SQ], val[:, vs], pT[:, 0:ncols],
                                         start=(kt == 0), stop=(kt == nkt - 1))
                        if d < 0:
                            # full tile: fold into the running z accumulator
                            if zacc is None:
                                zacc = pT
                            elif zacc is pT:  # never happens; keep linter quiet
                                pass
                            else:
                                if kt == 1:
                                    zs = att_t.tile([128, SQ], FP16, tag="zs", name="zs", bufs=2)
                                    nc.vector.tensor_add(zs[:], zacc[:], pT[:])
                                    zacc = zs
                                else:
                                    nc.vector.tensor_add(zacc[:], zacc[:], pT[:])
                            continue
                        if d == 0 and zacc is not None:
                            nc.tensor.matmul(pz[:], ones_h[:], zacc[:], start=True, stop=False)
                            z_started = True
                        nc.tensor.matmul(pz[:, off:SQ], ones_h[:], pT[:, 0:ncols],
                                         start=(not z_started), stop=(kt == nkt - 1))
                        z_started = True
                    rz = att_t.tile([128, SQ], FP32, tag="rz", name="rz", bufs=2)
                    nc.vector.reciprocal(rz[:], pz[:])
                    ao_h = att_t.tile([128, SQ], FP16, tag=f"ao{h}", name=f"ao{h}", bufs=2)
                    nc.vector.tensor_mul(ao_h[:], pav[:], rz[:])
                    ao.append(ao_h)

                prev_proj = (st, ao)

            emit_proj(*prev_proj)

        attn_cm.__exit__(None, None, None)
        ps_lin_cm.__exit__(None, None, None)
        dec_s_cm.__exit__(None, None, None)
        dec_w_cm.__exit__(None, None, None)
        nq_cm.__exit__(None, None, None)
        const_cm.__exit__(None, None, None)
        dram_cm.__exit__(None, None, None)

    nc.compile()
    return nc


def _dup_rope_rows(w64):
    # [64, n] rope rows (interleaved pairs) -> [128, n] = [x0; x0; x1; x1]
    x0, x1 = w64[0::2], w64[1::2]
    return np.concatenate([x0, x0, x1, x1], axis=0)


def _prep_inputs(x, freqs_cis, w_cq, w_qnorm, w_dqn, w_dqr, w_ckv, w_kvnorm, w_dkn, w_dv,
                 w_krope, w_proj):
    f16 = np.float16

    cos = freqs_cis[:, :, 0].T.astype(np.float32)   # (32, S)
    sin = freqs_cis[:, :, 1].T.astype(np.float32)
    ropeq = np.concatenate([cos, sin, -sin, cos], axis=0).astype(f16)  # (128, S)

    wcmpkv = np.concatenate([w_ckv.T, _dup_rope_rows(w_krope / H).T],
                            axis=1).astype(f16)      # (D, 640)
    wcmpq = w_cq.T.astype(f16)                       # (D, 512)

    wdqn = (w_dqn * w_qnorm[None, :] * SCALE).reshape(H, NOPE, QR)
    wdqr = (w_dqr * w_qnorm[None, :] * SCALE).reshape(H, ROPE, QR)
    wdkn = (w_dkn * w_kvnorm[None, :]).reshape(H, NOPE, KVR)
    wdv = (w_dv * w_kvnorm[None, :]).reshape(H, VH, KVR)
    wprojT_full = w_proj.T                           # (H*VH, D)

    tril = (np.arange(128)[None, :] >= np.arange(128)[:, None]).astype(f16)
    eye64 = np.concatenate([np.eye(64), np.eye(64)], axis=0).astype(f16)

    in_maps = []
    for c in range(W):
        g, r = divmod(c, GS)
        hs = slice(r * HPC, (r + 1) * HPC)
        pos = np.concatenate([np.arange(TBLK * r, TBLK * (r + 1)),
                              np.arange(1024 + TBLK * r, 1024 + TBLK * (r + 1))])
        xb = x[g].T  # (D, S)
        wdqr_dup = np.concatenate([_dup_rope_rows(wdqr[h]) for h in range(r * HPC, (r + 1) * HPC)],
                                  axis=0)            # (512, QR)
        wdec = np.concatenate([
            wdqn[hs].reshape(HPC * NOPE, QR),
            wdqr_dup,
            wdkn[hs].reshape(HPC * NOPE, KVR),
            wdv[hs].reshape(HPC * VH, KVR),
        ], axis=0).T.astype(f16)                     # (512, 2048)
        in_maps.append({
            "xT": np.ascontiguousarray(xb.astype(f16)),
            "xmyT": np.ascontiguousarray(xb[:, pos].astype(f16)),
            "wcmpkv": np.ascontiguousarray(wcmpkv),
            "wcmpq": np.ascontiguousarray(wcmpq),
            "wdec": np.ascontiguousarray(wdec),
            "wprojT": np.ascontiguousarray(wprojT_full[r * HPC * VH:(r + 1) * HPC * VH, :].astype(f16)),
            "ropeq": np.ascontiguousarray(ropeq),
            "ropek": np.ascontiguousarray(ropeq[:, pos]),
            "eye64": eye64,
            "tril": tril,
        })
    return in_maps


last_results = None


def kernel(x, mask, freqs_cis, w_cq, w_qnorm, w_dqn, w_dqr, w_ckv, w_kvnorm, w_dkn, w_dv,
           w_krope, w_proj):
    global last_results
    if "nc" not in _cache:
        _cache["nc"] = _build()
    nc = _cache["nc"]

    in_maps = _prep_inputs(np.asarray(x, np.float32), np.asarray(freqs_cis, np.float32),
                           np.asarray(w_cq, np.float32), np.asarray(w_qnorm, np.float32),
                           np.asarray(w_dqn, np.float32), np.asarray(w_dqr, np.float32),
                           np.asarray(w_ckv, np.float32), np.asarray(w_kvnorm, np.float32),
                           np.asarray(w_dkn, np.float32), np.asarray(w_dv, np.float32),
                           np.asarray(w_krope, np.float32), np.asarray(w_proj, np.float32))

    res = bass_utils.run_bass_kernel_spmd(nc, in_maps, core_ids=list(range(W)))
    last_results = res

    out = np.zeros((B, S, D), np.float32)
    for c in range(W):
        g, r = divmod(c, GS)
        oc = np.asarray(res.results[c]["out_c"], np.float32)  # (512, D)
        for st in range(NST):
            rows = slice(SQ * st + 128 * r, SQ * st + 128 * (r + 1))
            out[g, rows, :] = oc[128 * st:128 * (st + 1), :]
    return out


# revision 49
# speedup vs baseline: 1.0041x; 1.0041x over previous
"""MLA (multi-head latent attention) forward on 8 Trainium2 NeuronCores.

Sharding: 2 groups of 4 cores, one batch element per group. Within a group:
- kv+krope compress is token-parallel (2x256-token blocks/core), exchanged via
  two chunked AllGathers (576 rows x 256 tokens each) that overlap compute.
- q compress is replicated (each core compresses all 2048 batch tokens), so q
  latents never leave the core.
- decompress + attention are head-parallel (4 heads/core).
- output projection is computed as per-supertile partials over the core's own
  heads and combined with pipelined ReduceScatter(add) collectives.
RoPE via duplicated-weight columns (no repacking DMAs); causal diagonal
narrowing on attention tiles.
"""

import numpy as np

import concourse.bacc as bacc
import concourse.mybir as mybir
import concourse.tile as tile
from concourse import bass_utils

B, S, D = 2, 2048, 2048
H = 16
NOPE, ROPE, VH = 128, 64, 128
QR = KVR = 512
EPS = 1e-6
W = 8            # cores
GS = 4           # group size (cores per batch)
HPC = H // GS    # heads per core = 4
TBLK = 256       # compress block tokens
SQ = 512         # q supertile width
KT = 128         # key tile
NST = S // SQ    # 4 supertiles
NBLK = S // TBLK # 8 blocks per batch
SCALE = 1.0 / np.sqrt(NOPE + ROPE)
KVROWS = KVR + ROPE  # 576 latent rows shipped (kv 0:512, krope 512:576)

FP16 = mybir.dt.float16
FP32 = mybir.dt.float32

_cache = {}


def _build():
    nc = bacc.Bacc("TRN2", target_bir_lowering=False, debug=False)

    def din(name, shape, dt=FP16):
        return nc.dram_tensor(name, shape, dt, kind="ExternalInput").ap()

    xT = din("xT", [D, S])                 # full batch, feature-major
    xmyT = din("xmyT", [D, 2 * TBLK])      # my 2 blocks, feature-major
    wcmpkv = din("wcmpkv", [D, 640])       # w_ckvT | w_kropeT_dup
    wcmpq = din("wcmpq", [D, 512])         # w_cqT
    wdec = din("wdec", [QR, 2048])         # wdqnT | wdqr_dupT | wdknT | wdvT (4 heads)
    wprojT = din("wprojT", [HPC * VH, D])  # my heads' rows of w_proj.T
    ropeq = din("ropeq", [128, S])         # [cos;sin;-sin;cos] all positions
    ropek = din("ropek", [128, 2 * TBLK])  # same, my 2 blocks' positions
    eye64 = din("eye64", [128, 64])        # [I64; I64]
    tril = din("tril", [128, 128])         # tril[k,q] = (q >= k)
    out_c = nc.dram_tensor("out_c", [SQ, D], FP16, kind="ExternalOutput").ap()

    groups = [[0, 1, 2, 3], [4, 5, 6, 7]]

    with tile.TileContext(nc) as tc:
        dram_cm = tc.tile_pool(name="dram", bufs=1, space="DRAM")
        dram = dram_cm.__enter__()
        lat_a = dram.tile([KVROWS, TBLK], FP16, tag="lat_a", name="lat_a")
        lat_b = dram.tile([KVROWS, TBLK], FP16, tag="lat_b", name="lat_b")
        lat_ga = dram.tile([GS, KVROWS, TBLK], FP16, tag="lat_ga", name="lat_ga")
        lat_gb = dram.tile([GS, KVROWS, TBLK], FP16, tag="lat_gb", name="lat_gb")
        rs_in = dram.tile([NST, SQ, D], FP16, tag="rs_in", name="rs_in")
        rs_out = dram.tile([NST, 128, D], FP16, tag="rs_out", name="rs_out")

        const_cm = tc.tile_pool(name="const", bufs=1)
        const = const_cm.__enter__()
        ones_h = const.tile([128, 128], FP16, tag="ones_h", name="ones_h")
        nc.any.memset(ones_h[:], 1.0)
        ones1 = const.tile([1, 128], FP16, tag="ones1", name="ones1")
        nc.any.memset(ones1[:], 1.0)
        invn = const.tile([128, 1], FP16, tag="invn", name="invn")
        nc.any.memset(invn[:], 1.0 / QR)
        eps_t = const.tile([1, 1], FP32, tag="eps_t", name="eps_t")
        nc.any.memset(eps_t[:], EPS)
        tril_t = const.tile([128, 128], FP16, tag="tril_t", name="tril_t")
        eye_t = const.tile([128, 64], FP16, tag="eye_t", name="eye_t")
        ropeq_t = const.tile([128, S], FP16, tag="ropeq_t", name="ropeq_t")
        ropek_t = const.tile([128, 2 * TBLK], FP16, tag="ropek_t", name="ropek_t")

        # persistent q latents (replicated compress)
        nq_cm = tc.tile_pool(name="nq", bufs=1)
        nqp = nq_cm.__enter__()
        nq_all = [nqp.tile([128, S], FP16, tag=f"nq{k}", name=f"nq{k}") for k in range(4)]

        # dec-phase pools allocated BEFORE compress pools so their SBUF/PSUM
        # space is disjoint (avoids false WAR waits on compress tiles)
        dec_w_cm = tc.tile_pool(name="dec_w", bufs=1)
        dec_w = dec_w_cm.__enter__()
        dec_s_cm = tc.tile_pool(name="dec_s", bufs=1)
        dec_s = dec_s_cm.__enter__()
        ps_lin_cm = tc.tile_pool(name="ps_lin", bufs=2, space="PSUM")
        ps_lin = ps_lin_cm.__enter__()
        wd = [dec_w.tile([128, 2048], FP16, tag=f"wd{k}", name=f"wd{k}") for k in range(4)]
        wp = [dec_w.tile([128, D], FP16, tag=f"wp{k}", name=f"wp{k}") for k in range(4)]

        # ---------------- Phase 1: compress ------------------------------------
        with tc.tile_pool(name="cmp_w", bufs=1) as cmp_w, \
             tc.tile_pool(name="cmp_t", bufs=1) as cmp_t, \
             tc.tile_pool(name="ps_cmp", bufs=1, space="PSUM") as ps_cmp:
            # kv path needs wckv and xmy first -- load those on SP.
            wckv = cmp_w.tile([128, 16 * 640], FP16, tag="wckv", name="wckv")
            xmy = cmp_w.tile([128, 16 * 2 * TBLK], FP16, tag="xmy", name="xmy")
            for hh in range(8):
                ks = slice(hh * 2 * 640, (hh + 1) * 2 * 640)
                nc.sync.dma_start(wckv[:, ks].rearrange("p (k c) -> p k c", k=2),
                                  wcmpkv[hh * 256:(hh + 1) * 256, :].rearrange("(k p) c -> p k c", p=128))
                ks = slice(hh * 2 * 512, (hh + 1) * 2 * 512)
                nc.sync.dma_start(xmy[:, ks].rearrange("p (k t) -> p k t", k=2),
                                  xmyT[hh * 256:(hh + 1) * 256, :].rearrange("(k p) t -> p k t", p=128))
                if hh == 0:  # const tables deferred behind the critical kv pieces
                    nc.sync.dma_start(ropek_t[:], ropek[:])
                    nc.sync.dma_start(eye_t[:], eye64[:])
                    nc.sync.dma_start(tril_t[:], tril[:])
                    nc.sync.dma_start(ropeq_t[:], ropeq[:])
            wcq = cmp_w.tile([128, 16 * 512], FP16, tag="wcq", name="wcq")
            xtc = {}

            def load_xt(cc):
                t = cmp_w.tile([128, 16 * 512], FP16, tag="xt", name=f"xt{cc}", bufs=2)
                xtc[cc] = t
                for hh in range(4):
                    ks = slice(hh * 4 * 512, (hh + 1) * 4 * 512)
                    nc.sync.dma_start(
                        t[:, ks].rearrange("p (k t) -> p k t", k=4),
                        xT[hh * 512:(hh + 1) * 512, cc * 512:(cc + 1) * 512].rearrange(
                            "(k p) t -> p k t", p=128))

            load_xt(0)
            for hh in range(4):
                ks = slice(hh * 4 * 512, (hh + 1) * 4 * 512)
                nc.sync.dma_start(wcq[:, ks].rearrange("p (k c) -> p k c", k=4),
                                  wcmpq[hh * 512:(hh + 1) * 512, :].rearrange("(k p) c -> p k c", p=128))


            def rmsnorm(ps4, dst_fn):
                """ps4: 4 psum APs [128, TBLK]; dst_fn(m) -> fp16 dest AP."""
                cq = cmp_t.tile([128, 4 * TBLK], FP32, tag="cq", name="cq", bufs=2)
                sq = cmp_t.tile([128, 4 * TBLK], FP16, tag="sq", name="sq", bufs=2)
                for m in range(4):
                    cs = slice(m * TBLK, (m + 1) * TBLK)
                    nc.scalar.activation(cq[:, cs], ps4[m], mybir.ActivationFunctionType.Copy)
                    nc.vector.tensor_mul(sq[:, cs], cq[:, cs], cq[:, cs])
                # one psum bank: ssq in [0:1, 0:TBLK], rstd broadcast in [:, TBLK:2*TBLK]
                ps_nrm = ps_cmp.tile([128, 2 * TBLK], FP32, tag="nrm", name="nrm", bufs=1)
                for m in range(4):
                    nc.tensor.matmul(ps_nrm[0:1, 0:TBLK], invn[:], sq[:, m * TBLK:(m + 1) * TBLK],
                                     start=(m == 0), stop=(m == 3))
                std_f = cmp_t.tile([1, TBLK], FP32, tag="std", name="std", bufs=2)
                nc.scalar.activation(std_f[:], ps_nrm[0:1, 0:TBLK], mybir.ActivationFunctionType.Sqrt,
                                     bias=eps_t[:])
                rstd_f = cmp_t.tile([1, TBLK], FP32, tag="rstdf", name="rstdf", bufs=2)
                nc.vector.reciprocal(rstd_f[:], std_f[:])
                rstd = cmp_t.tile([1, TBLK], FP16, tag="rstd", name="rstd", bufs=2)
                nc.vector.tensor_copy(rstd[:], rstd_f[:])
                nc.tensor.matmul(ps_nrm[:, TBLK:2 * TBLK], ones1[:], rstd[:], start=True, stop=True)
                for m in range(4):
                    nc.vector.tensor_mul(dst_fn(m), cq[:, m * TBLK:(m + 1) * TBLK],
                                         ps_nrm[:, TBLK:2 * TBLK])

            # kv+krope for my 2 blocks -> AllGather chunks
            def kv_block(half):
                lat_dst, lat_g = ((lat_a, lat_ga), (lat_b, lat_gb))[half]
                xoff = half * TBLK  # column offset inside my 2-block slice (input-dependent)
                psk = [ps_cmp.tile([128, TBLK], FP32, tag=f"cmk{m}", name=f"cmk{m}")[:]
                       for m in range(4)]
                psr = ps_cmp.tile([128, TBLK], FP32, tag="cmr", name="cmr")[:]
                for k in range(16):
                    xk = xmy[:, k * 2 * TBLK + xoff: k * 2 * TBLK + xoff + TBLK]
                    wb = k * 640
                    for m in range(4):
                        nc.tensor.matmul(psk[m], wckv[:, wb + m * 128: wb + (m + 1) * 128],
                                         xk, start=(k == 0), stop=(k == 15))
                    nc.tensor.matmul(psr, wckv[:, wb + 512: wb + 640],
                                     xk, start=(k == 0), stop=(k == 15))

                lat_s = cmp_t.tile([128, 4 * TBLK], FP16, tag="lat_s", name="lat_s", bufs=2)
                rmsnorm(psk, lambda m: lat_s[:, m * TBLK:(m + 1) * TBLK])
                nc.sync.dma_start(
                    lat_dst[0:KVR, :].rearrange("(m p) c -> p m c", p=128),
                    lat_s[:].rearrange("p (m c) -> p m c", m=4))
                # krope: rope via dup layout, fold 128->64 rows with [I;I] matmul
                ktmp = cmp_t.tile([128, TBLK], FP16, tag="ktmp", name="ktmp", bufs=2)
                nc.vector.tensor_mul(ktmp[:], psr,
                                     ropek_t[:, half * TBLK:(half + 1) * TBLK])
                ps_nrm_f = ps_cmp.tile([128, 2 * TBLK], FP32, tag="nrm", name="nrm", bufs=1)
                ps_kr = ps_nrm_f[64:128, 0:TBLK]
                nc.tensor.matmul(ps_kr, eye_t[:], ktmp[:], start=True, stop=True)
                kr16 = cmp_t.tile([64, TBLK], FP16, tag="kr16", name="kr16", bufs=2)
                nc.scalar.activation(kr16[:], ps_kr, mybir.ActivationFunctionType.Copy)
                nc.sync.dma_start(lat_dst[KVR:KVROWS, :], kr16[:])

                nc.gpsimd.collective_compute(
                    "AllGather",
                    mybir.AluOpType.bypass,
                    ins=[lat_dst[:].rearrange("a b -> (a b)")],
                    outs=[lat_g[:].rearrange("w a b -> (w a b)")],
                    replica_groups=groups,
                )

            kv_block(0)
            load_xt(1)
            kv_block(1)
            for k in range(4):
                nc.sync.dma_start(wd[k][:], wdec[k * 128:(k + 1) * 128, :])
            for k in range(4):
                nc.sync.dma_start(wp[k][:], wprojT[k * 128:(k + 1) * 128, :])

            # q compress for ALL 8 blocks (replicated) -> nq_all sbuf
            for blk in range(NBLK):
                if blk in (2, 4) and blk // 2 + 1 < 4:
                    load_xt(blk // 2 + 1)
                bs = slice(blk * TBLK, (blk + 1) * TBLK)
                xsrc = xtc[blk // 2]
                xo = (blk % 2) * TBLK
                psq = [ps_cmp.tile([128, TBLK], FP32, tag=f"cmk{m}", name=f"cmk{m}")[:]
                       for m in range(4)]
                for k in range(16):
                    xk = xsrc[:, k * 512 + xo: k * 512 + xo + TBLK]
                    for m in range(4):
                        nc.tensor.matmul(psq[m], wcq[:, k * 512 + m * 128: k * 512 + (m + 1) * 128],
                                         xk, start=(k == 0), stop=(k == 15))
                rmsnorm(psq, lambda m, bs=bs: nq_all[m][:, bs])

        # ---------------- Phase 2: decompress + attention + partial proj ---------
        attn_cm = tc.tile_pool(name="attn", bufs=1)
        ap_ = attn_cm.__enter__()
        qn = [ap_.tile([128, S], FP16, tag=f"qn{h}", name=f"qn{h}") for h in range(HPC)]
        qr_ = [ap_.tile([128, S], FP16, tag=f"qr{h}", name=f"qr{h}") for h in range(HPC)]
        kn = [ap_.tile([128, S], FP16, tag=f"kn{h}", name=f"kn{h}") for h in range(HPC)]
        krg2 = [dec_s.tile([128, SQ], FP16, tag=f"krg2{st}", name=f"krg2{st}")
                for st in range(NST)]
        val = ap_.tile([128, 16 * 512], FP16, tag="val", name="val")

        with tc.tile_pool(name="ps_s", bufs=2, space="PSUM") as ps_s, \
             tc.tile_pool(name="ps_av", bufs=2, space="PSUM") as ps_av, \
             tc.tile_pool(name="ps_z", bufs=2, space="PSUM") as ps_z, \
             tc.tile_pool(name="att_t", bufs=1) as att_t:
            # preload ALL kv latent chunks up front (SP blocks only on the AGs)
            nkv_all = []
            for st in range(NST):
                lat_g = lat_ga if st < 2 else lat_gb
                j0 = (st % 2) * 2
                ss = slice(st * SQ, (st + 1) * SQ)
                nkv_t = []
                for k in range(4):
                    t_ = dec_s.tile([128, SQ], FP16, tag=f"nkv{st}{k}", name=f"nkv{st}{k}")
                    nc.sync.dma_start(
                        t_[:].rearrange("p (j c) -> p j c", j=2),
                        lat_g[j0:j0 + 2, k * 128:(k + 1) * 128, :].rearrange("j p c -> p j c"))
                    nkv_t.append(t_)
                nkv_all.append(nkv_t)
                nc.sync.dma_start(
                    krg2[st][0:64, :].rearrange("p (j c) -> p j c", j=2),
                    lat_g[j0:j0 + 2, KVR:KVROWS, :].rearrange("j p c -> p j c"))
                nc.sync.dma_start(
                    krg2[st][64:128, :].rearrange("p (j c) -> p j c", j=2),
                    lat_g[j0:j0 + 2, KVR:KVROWS, :].rearrange("j p c -> p j c"))

            def emit_rs(st):
                nc.gpsimd.collective_compute(
                    "ReduceScatter",
                    mybir.AluOpType.add,
                    ins=[rs_in[st].rearrange("a b -> (a b)")],
                    outs=[rs_out[st].rearrange("a b -> (a b)")],
                    replica_groups=groups,
                )
                nc.sync.dma_start(out_c[st * 128:(st + 1) * 128, :], rs_out[st])

            def emit_proj(st, ao):
                # partial output projection for supertile st's tokens
                stage = att_t.tile([128, 4 * D], FP16, tag="stage", name="stage", bufs=2)
                for tch in range(4):
                    tsl = slice(tch * 128, (tch + 1) * 128)
                    for dch in range(4):
                        ps = ps_lin.tile([128, 512], FP32, tag="lin", name="lin")
                        for h in range(HPC):
                            nc.tensor.matmul(ps[:], ao[h][:, tsl],
                                             wp[h][:, dch * 512:(dch + 1) * 512],
                                             start=(h == 0), stop=(h == 3))
                        if (tch + dch) % 2 == 0:
                            nc.scalar.activation(stage[:, tch * D + dch * 512: tch * D + (dch + 1) * 512],
                                                 ps[:], mybir.ActivationFunctionType.Copy)
                        else:
                            nc.vector.tensor_copy(stage[:, tch * D + dch * 512: tch * D + (dch + 1) * 512],
                                                  ps[:])
                for hh in range(4):
                    nc.sync.dma_start(
                        rs_in[st, hh * 128:(hh + 1) * 128, :],
                        stage[:, hh * D:(hh + 1) * D])
                emit_rs(st)

            prev_proj = None
            for st in range(NST):
                ss = slice(st * SQ, (st + 1) * SQ)
                nkv_t = nkv_all[st]

                # q decompress from local latents (no AG dependency)
                for h in range(HPC):
                    ps = ps_lin.tile([128, SQ], FP32, tag="lin", name="lin")
                    for k in range(4):
                        nc.tensor.matmul(ps[:], wd[k][:, h * 128:(h + 1) * 128],
                                         nq_all[k][:, ss], start=(k == 0), stop=(k == 3))
                    nc.vector.tensor_copy(qn[h][:, ss], ps[:])
                for h in range(HPC):
                    ps = ps_lin.tile([128, SQ], FP32, tag="lin", name="lin")
                    for k in range(4):
                        nc.tensor.matmul(ps[:], wd[k][:, 512 + h * 128: 512 + (h + 1) * 128],
                                         nq_all[k][:, ss], start=(k == 0), stop=(k == 3))
                    nc.vector.tensor_mul(qr_[h][:, ss], ps[:], ropeq_t[:, ss])

                # k/v decompress
                for h in range(HPC):
                    ps = ps_lin.tile([128, SQ], FP32, tag="lin", name="lin")
                    for k in range(4):
                        nc.tensor.matmul(ps[:], wd[k][:, 1024 + h * 128: 1024 + (h + 1) * 128],
                                         nkv_t[k][:], start=(k == 0), stop=(k == 3))
                    nc.scalar.activation(kn[h][:, ss], ps[:], mybir.ActivationFunctionType.Copy)
                for j in range(4):
                    ps = ps_lin.tile([128, SQ], FP32, tag="lin", name="lin")
                    for k in range(4):
                        nc.tensor.matmul(ps[:], nkv_t[k][:, j * 128:(j + 1) * 128],
                                         wd[k][:, 1536:2048], start=(k == 0), stop=(k == 3))
                    ktg = st * 4 + j
                    nc.vector.tensor_copy(val[:, ktg * 512:(ktg + 1) * 512], ps[:])

                # previous supertile's projection (hides softmax-normalize latency)
                if prev_proj is not None:
                    emit_proj(*prev_proj)

                # attention (Q = st), 4 heads
                nkt = 4 * (st + 1)
                ao = []
                for h in range(HPC):
                    pav = ps_av.tile([128, SQ], FP32, tag="av", name="av")
                    pz = ps_z.tile([128, SQ], FP32, tag="z", name="z")
                    z_started = False
                    zacc = None  # running sum of full-tile pT's
                    for kt in range(nkt):
                        d = kt - 4 * st
                        off = max(0, d) * 128
                        ncols = SQ - off
                        ks = slice(kt * KT, (kt + 1) * KT)
                        qs = slice(st * SQ + off, (st + 1) * SQ)
                        pss = ps_s.tile([128, SQ], FP32, tag="s", name="s")
                        nc.tensor.matmul(pss[:, 0:ncols], kn[h][:, ks], qn[h][:, qs],
                                         start=True, stop=False)
                        nc.tensor.matmul(pss[:, 0:ncols], krg2[kt // 4][:, (kt % 4) * KT:(kt % 4 + 1) * KT], qr_[h][:, qs],
                                         start=False, stop=True)
                        pT = att_t.tile([128, SQ], FP16, tag="pT", name="pT", bufs=6)
                        nc.scalar.activation(pT[:, 0:ncols], pss[:, 0:ncols],
                                             mybir.ActivationFunctionType.Exp)
                        if d >= 0:
                            nc.vector.tensor_mul(pT[:, 0:128], pT[:, 0:128], tril_t[:])
                        vs = slice(kt * 512 + h * 128, kt * 512 + (h + 1) * 128)
                        nc.tensor.matmul(pav[:, off:SQ], val[:, vs], pT[:, 0:ncols],
                                         start=(kt == 0), stop=(kt == nkt - 1))
                        if d < 0:
                            # full tile: fold into the running z accumulator
                            if zacc is None:
                                zacc = pT
                            elif zacc is pT:  # never happens; keep linter quiet
                                pass
                            else:
                                if kt == 1:
                                    zs = att_t.tile([128, SQ], FP16, tag="zs", name="zs", bufs=2)
                                    nc.vector.tensor_add(zs[:], zacc[:], pT[:])
                                    zacc = zs
                                else:
                                    nc.vector.tensor_add(zacc[:], zacc[:], pT[:])
                            continue
                        if d == 0 and zacc is not None:
                            nc.tensor.matmul(pz[:], ones_h[:], zacc[:], start=True, stop=False)
                            z_started = True
                        nc.tensor.matmul(pz[:, off:SQ], ones_h[:], pT[:, 0:ncols],
                                         start=(not z_started), stop=(kt == nkt - 1))
                        z_started = True
                    rz = att_t.tile([128, SQ], FP32, tag="rz", name="rz", bufs=2)
                    nc.vector.reciprocal(rz[:], pz[:])
                    ao_h = att_t.tile([128, SQ], FP16, tag=f"ao{h}", name=f"ao{h}", bufs=2)
                    nc.vector.tensor_mul(ao_h[:], pav[:], rz[:])
                    ao.append(ao_h)

                prev_proj = (st, ao)

            emit_proj(*prev_proj)

        attn_cm.__exit__(None, None, None)
        ps_lin_cm.__exit__(None, None, None)
        dec_s_cm.__exit__(None, None, None)
        dec_w_cm.__exit__(None, None, None)
        nq_cm.__exit__(None, None, None)
        const_cm.__exit__(None, None, None)
        dram_cm.__exit__(None, None, None)

    nc.compile()
    return nc


def _dup_rope_rows(w64):
    # [64, n] rope rows (interleaved pairs) -> [128, n] = [x0; x0; x1; x1]
    x0, x1 = w64[0::2], w64[1::2]
    return np.concatenate([x0, x0, x1, x1], axis=0)


def _prep_inputs(x, freqs_cis, w_cq, w_qnorm, w_dqn, w_dqr, w_ckv, w_kvnorm, w_dkn, w_dv,
                 w_krope, w_proj):
    f16 = np.float16

    cos = freqs_cis[:, :, 0].T.astype(np.float32)   # (32, S)
    sin = freqs_cis[:, :, 1].T.astype(np.float32)
    ropeq = np.concatenate([cos, sin, -sin, cos], axis=0).astype(f16)  # (128, S)

    wcmpkv = np.concatenate([w_ckv.T, _dup_rope_rows(w_krope / H).T],
                            axis=1).astype(f16)      # (D, 640)
    wcmpq = w_cq.T.astype(f16)                       # (D, 512)

    wdqn = (w_dqn * w_qnorm[None, :] * SCALE).reshape(H, NOPE, QR)
    wdqr = (w_dqr * w_qnorm[None, :] * SCALE).reshape(H, ROPE, QR)
    wdkn = (w_dkn * w_kvnorm[None, :]).reshape(H, NOPE, KVR)
    wdv = (w_dv * w_kvnorm[None, :]).reshape(H, VH, KVR)
    wprojT_full = w_proj.T                           # (H*VH, D)

    tril = (np.arange(128)[None, :] >= np.arange(128)[:, None]).astype(f16)
    eye64 = np.concatenate([np.eye(64), np.eye(64)], axis=0).astype(f16)

    in_maps = []
    for c in range(W):
        g, r = divmod(c, GS)
        hs = slice(r * HPC, (r + 1) * HPC)
        pos = np.concatenate([np.arange(TBLK * r, TBLK * (r + 1)),
                              np.arange(1024 + TBLK * r, 1024 + TBLK * (r + 1))])
        xb = x[g].T  # (D, S)
        wdqr_dup = np.concatenate([_dup_rope_rows(wdqr[h]) for h in range(r * HPC, (r + 1) * HPC)],
                                  axis=0)            # (512, QR)
        wdec = np.concatenate([
            wdqn[hs].reshape(HPC * NOPE, QR),
            wdqr_dup,
            wdkn[hs].reshape(HPC * NOPE, KVR),
            wdv[hs].reshape(HPC * VH, KVR),
        ], axis=0).T.astype(f16)                     # (512, 2048)
        in_maps.append({
            "xT": np.ascontiguousarray(xb.astype(f16)),
            "xmyT": np.ascontiguousarray(xb[:, pos].astype(f16)),
            "wcmpkv": np.ascontiguousarray(wcmpkv),
            "wcmpq": np.ascontiguousarray(wcmpq),
            "wdec": np.ascontiguousarray(wdec),
            "wprojT": np.ascontiguousarray(wprojT_full[r * HPC * VH:(r + 1) * HPC * VH, :].astype(f16)),
            "ropeq": np.ascontiguousarray(ropeq),
            "ropek": np.ascontiguousarray(ropeq[:, pos]),
            "eye64": eye64,
            "tril": tril,
        })
    return in_maps


last_results = None


def kernel(x, mask, freqs_cis, w_cq, w_qnorm, w_dqn, w_dqr, w_ckv, w_kvnorm, w_dkn, w_dv,
           w_krope, w_proj):
    global last_results
    if "nc" not in _cache:
        _cache["nc"] = _build()
    nc = _cache["nc"]

    in_maps = _prep_inputs(np.asarray(x, np.float32), np.asarray(freqs_cis, np.float32),
                           np.asarray(w_cq, np.float32), np.asarray(w_qnorm, np.float32),
                           np.asarray(w_dqn, np.float32), np.asarray(w_dqr, np.float32),
                           np.asarray(w_ckv, np.float32), np.asarray(w_kvnorm, np.float32),
                           np.asarray(w_dkn, np.float32), np.asarray(w_dv, np.float32),
                           np.asarray(w_krope, np.float32), np.asarray(w_proj, np.float32))

    res = bass_utils.run_bass_kernel_spmd(nc, in_maps, core_ids=list(range(W)))
    last_results = res

    out = np.zeros((B, S, D), np.float32)
    for c in range(W):
        g, r = divmod(c, GS)
        oc = np.asarray(res.results[c]["out_c"], np.float32)  # (512, D)
        for st in range(NST):
            rows = slice(SQ * st + 128 * r, SQ * st + 128 * (r + 1))
            out[g, rows, :] = oc[128 * st:128 * (st + 1), :]
    return out
